# revision 2
# baseline (speedup 1.0000x reference)
"""RGCN 2-layer kernel v3 for 8 TRN2 NeuronCores (Bass/Tile).

Three launches, dst-sharded (core k owns nodes [12500k, 12500(k+1))):

A) stage0: xt[r] = W1_r^T x_k^T on device (each core its node slice)
   -> host gathers per-edge 64-ch messages msgs[e] = xt[et_e][:, src_e].

B) layer-1 scatter: windows of 128 dst nodes; two edges sharing a dst
   packed per slot ([mA|mB] in a [128,128] bf16 tile -> FWL); one-hot S
   (fp8) scatters slots into node columns; PSUM rows 0:64 + 64:128 are
   the two halves, summed on evacuation; + root matmul + bias + relu.

C) layer-2 aggregate-first: windows of 1024 rel-major (node,rel)
   segments; two edges of the same segment packed per slot; stage 2
   applies stacked [W2_r;W2_r] (contraction 128) on contiguous rel
   slices, grouped over GG windows; + root + bias.

Mean-norm (per (dst,rel)) folded into the gathered messages on host.
Per-core node permutations balance slot counts across windows.
"""
import sys
import numpy as np

if '/opt/trn_rl_repo' not in sys.path:
    sys.path.insert(0, '/opt/trn_rl_repo')

import ml_dtypes  # noqa: E402
import concourse.bacc as bacc  # noqa: E402
import concourse.mybir as mybir  # noqa: E402
import concourse.tile as tile  # noqa: E402
from concourse.bass_utils import run_bass_kernel_spmd  # noqa: E402

BF16 = mybir.dt.bfloat16
FP8 = mybir.dt.float8e4
F32 = mybir.dt.float32
BF = ml_dtypes.bfloat16
F8 = ml_dtypes.float8_e4m3

N_NODES = 100000
N_REL = 8
N_CORES = 8
NPC = N_NODES // N_CORES
P = 128
N_WIN = -(-NPC // P)              # 98
NPC_PAD = N_WIN * P               # 12544
HID = 64
WIN = 1024                        # segs per window in launch C
GG = 4                            # stage-2 window grouping (launch C)
SPAN = 128                        # one-hot block span (launch C)
STG = 64                          # block base stagger (launch C)
N_BASE = (WIN - SPAN) // STG


def _block_base(i, t_w):
    return STG * min(N_BASE, (i * (N_BASE + 1)) // (t_w - 1))


def _block_base_vec(slot_tile, t_w):
    i = slot_tile % t_w
    return STG * np.minimum(N_BASE, (i * (N_BASE + 1)) // (t_w - 1))


def assign_slots(seg_local, t_w):
    n_grp = WIN // STG
    counts = np.bincount(seg_local // STG, minlength=n_grp)
    fill = [0] * t_w
    pieces = []
    j = 0
    for g in range(n_grp):
        c = int(counts[g])
        while c > 0:
            while j < t_w and (fill[j] >= P or
                               _block_base(j, t_w) + SPAN < STG * g + STG):
                j += 1
            if j >= t_w or _block_base(j, t_w) > STG * g:
                return None
            take = min(c, P - fill[j])
            pieces.append((g, j, take))
            fill[j] += take
            c -= take
    tile_idx = np.empty(len(seg_local), dtype=np.int32)
    pos = 0
    for (g, j, take) in pieces:
        tile_idx[pos:pos + take] = j
        pos += take
    return tile_idx


def prep_win_slots(seg, t_w, win):
    """seg: per-slot window-global segment ids, sorted. win: segs/window.
    Returns (slot_tile, slot_pos) global tile id + partition per slot."""
    n = len(seg)
    slot_tile = np.empty(n, dtype=np.int32)
    slot_pos = np.empty(n, dtype=np.int32)
    bounds = np.searchsorted(seg, np.arange(0, N_WIN * win + 1, win))
    for w in range(N_WIN):
        a, b = bounds[w], bounds[w + 1]
        if a == b:
            continue
        if win == WIN:
            ti = assign_slots(seg[a:b] - w * win, t_w)
            if ti is None:
                return None
        else:  # win == P: no block constraint, fill sequentially
            if b - a > t_w * P:
                return None
            ti = (np.arange(b - a) // P).astype(np.int32)
        order = np.argsort(ti, kind='stable')
        tlocal = ti[order]
        posl = np.empty(b - a, dtype=np.int32)
        uniq, start_idx = np.unique(tlocal, return_index=True)
        for u, s0 in zip(uniq, start_idx):
            cnt = int((tlocal == u).sum())
            posl[s0:s0 + cnt] = np.arange(cnt)
        st = np.empty(b - a, dtype=np.int32)
        sp = np.empty(b - a, dtype=np.int32)
        st[order] = w * t_w + tlocal
        sp[order] = posl
        slot_tile[a:b] = st
        slot_pos[a:b] = sp
    return slot_tile, slot_pos


def balance_windows(load_node):
    """LPT-balance nodes into windows by per-node load.
    Returns perm[NPC_PAD] (newpos -> old local id, -1 = pad)."""
    import heapq
    order = np.argsort(-load_node, kind='stable')
    heap = [(0, 0, w) for w in range(N_WIN)]
    heapq.heapify(heap)
    members = [[] for _ in range(N_WIN)]
    for n in order:
        while True:
            load, cnt, w = heapq.heappop(heap)
            if cnt < P:
                break
        members[w].append(n)
        heapq.heappush(heap, (load + int(load_node[n]), cnt + 1, w))
    perm = np.full(NPC_PAD, -1, dtype=np.int64)
    for w in range(N_WIN):
        m = members[w]
        perm[w * P:w * P + len(m)] = m
    return perm


def pair_up(seg_sorted):
    """Pair consecutive same-seg entries. Returns (slot_of_edge, half,
    slot_seg): slot ids ordered by (seg, pair)."""
    n = len(seg_sorted)
    change = np.r_[True, seg_sorted[1:] != seg_sorted[:-1]]
    segstart = np.maximum.accumulate(np.where(change, np.arange(n), 0))
    posin = np.arange(n) - segstart
    half = (posin % 2).astype(np.int8)
    pairidx = posin // 2
    key = seg_sorted.astype(np.int64) * 256 + pairidx
    uniq, slot_of_edge = np.unique(key, return_inverse=True)
    slot_seg = (uniq // 256).astype(np.int64)
    return slot_of_edge, half, slot_seg


# ---------------- device programs ----------------

def build_stage0_nc(reps=1, bch=4):
    """xtO[p, c, (r,hid)] = xt for node c*128+p: one x-stationary matmul
    (lhsT = x chunk [128, 128] -> FWL) against all rels' W at once."""
    RH = N_REL * HID
    nc = bacc.Bacc(None, target_bir_lowering=False)
    W = nc.dram_tensor("W", [P, RH], BF16, kind="ExternalInput")
    xT = nc.dram_tensor("xT", [P, NPC_PAD], BF16, kind="ExternalInput")
    xtO = nc.dram_tensor("xtO", [P, N_WIN, RH], BF16,
                         kind="ExternalOutput")
    with tile.TileContext(nc) as tc:
        with tc.tile_pool(name="wp", bufs=1) as wp, \
             tc.tile_pool(name="xp", bufs=1) as xp, \
             tc.tile_pool(name="op", bufs=3) as op, \
             tc.tile_pool(name="ps", bufs=4, space="PSUM") as ps:
            w_t = wp.tile([P, RH], BF16)
            nc.sync.dma_start(out=w_t[:], in_=W[:])
            x_t = xp.tile([P, NPC_PAD], BF16)
            nc.sync.dma_start(out=x_t[:], in_=xT[:])

            def _body():
                for c0 in range(0, N_WIN, bch):
                    nb = min(bch, N_WIN - c0)
                    o_t = op.tile([P, bch * RH], BF16, tag="o")
                    for b in range(nb):
                        c = c0 + b
                        p_t = ps.tile([P, RH], F32, tag="ps")
                        nc.tensor.matmul(
                            out=p_t[:],
                            lhsT=x_t[:, c * P:(c + 1) * P],
                            rhs=w_t[:], start=True, stop=True)
                        if c % 2 == 0:
                            nc.scalar.copy(out=o_t[:, b * RH:(b + 1) * RH],
                                           in_=p_t[:])
                        else:
                            nc.vector.tensor_copy(
                                out=o_t[:, b * RH:(b + 1) * RH], in_=p_t[:])
                    nc.sync.dma_start(
                        out=xtO[:, c0:c0 + nb, :].rearrange(
                            "p c f -> p (c f)"),
                        in_=o_t[:, :nb * RH])

            if reps == 1:
                _body()
            else:
                with tc.For_i(0, reps, 1):
                    _body()
    nc.compile()
    return nc


def build_l1_nc(t_w, dg=4, reps=1, dma_only=False, pe_only=False):
    """Launch B: paired scatter into node windows + root + bias + relu."""
    n_tiles = N_WIN * t_w
    nc = bacc.Bacc(None, target_bir_lowering=False)
    G = nc.dram_tensor("G", [P, n_tiles, P], BF16, kind="ExternalInput")
    S = nc.dram_tensor("S", [P, n_tiles, P], FP8, kind="ExternalInput")
    rootp = nc.dram_tensor("rootp", [P, P], BF16, kind="ExternalInput")
    bias = nc.dram_tensor("bias", [HID, 1], F32, kind="ExternalInput")
    xT = nc.dram_tensor("xT", [P, NPC_PAD], BF16, kind="ExternalInput")
    outT = nc.dram_tensor("outT", [HID, NPC_PAD], BF16,
                          kind="ExternalOutput")
    with tile.TileContext(nc) as tc:
        with tc.tile_pool(name="gs", bufs=3) as gs_pool, \
             tc.tile_pool(name="wp", bufs=1) as wp, \
             tc.tile_pool(name="ev", bufs=3) as ev, \
             tc.tile_pool(name="ps", bufs=4, space="PSUM") as ps:
            root_t = wp.tile([P, P], BF16)
            nc.sync.dma_start(out=root_t[:], in_=rootp[:])
            bias_t = wp.tile([HID, 1], F32)
            nc.sync.dma_start(out=bias_t[:], in_=bias[:])
            xT_t = wp.tile([P, NPC_PAD], BF16)
            nc.sync.dma_start(out=xT_t[:], in_=xT[:])

            def _body():
              for w0 in range(0, N_WIN, dg):
                nw = min(dg, N_WIN - w0)
                t0 = w0 * t_w
                g_t = gs_pool.tile([P, nw * t_w * P], BF16, tag="g")
                s_t = gs_pool.tile([P, nw * t_w * P], FP8, tag="s")
                if not pe_only:
                    nc.sync.dma_start(
                        out=g_t[:], in_=G[:, t0:t0 + nw * t_w, :].rearrange(
                            "p t c -> p (t c)"))
                    nc.scalar.dma_start(
                        out=s_t[:], in_=S[:, t0:t0 + nw * t_w, :].rearrange(
                            "p t c -> p (t c)"))
                if dma_only:
                    nc.sync.dma_start(
                        out=outT[:, w0 * P:(w0 + nw) * P],
                        in_=g_t[:HID, :nw * P])
                    continue
                # One psum tile for the dg windows; each window's chain
                # (scatter MMs + its root MM, with stop) is contiguous and
                # confined to its own 128-col region before the next
                # window's start - the device-validated pattern.
                a_ps = ps.tile([P, dg * P], F32, tag="aps")
                for dw in range(nw):
                    w = w0 + dw
                    reg = a_ps[:, dw * P:(dw + 1) * P]
                    for i in range(t_w):
                        j = dw * t_w + i
                        nc.tensor.matmul(
                            out=reg,
                            lhsT=g_t[:, j * P:(j + 1) * P],
                            rhs=s_t[:, j * P:(j + 1) * P],
                            start=(i == 0), stop=False)
                    nc.tensor.matmul(
                        out=reg, lhsT=root_t[:],
                        rhs=xT_t[:, w * P:(w + 1) * P],
                        start=False, stop=True)
                hi_t = ev.tile([HID, nw * P], F32, tag="hi")
                nc.scalar.copy(out=hi_t[:], in_=a_ps[HID:, :nw * P])
                sum_t = ev.tile([HID, nw * P], F32, tag="sum")
                nc.vector.tensor_tensor(
                    out=sum_t[:], in0=a_ps[:HID, :nw * P], in1=hi_t[:],
                    op=mybir.AluOpType.add)
                h_t = ev.tile([HID, nw * P], BF16, tag="h")
                nc.scalar.activation(
                    out=h_t[:], in_=sum_t[:],
                    func=mybir.ActivationFunctionType.Relu,
                    bias=bias_t[:])
                if not pe_only:
                    nc.sync.dma_start(
                        out=outT[:, w0 * P:(w0 + nw) * P], in_=h_t[:])

            if reps == 1:
                _body()
            else:
                with tc.For_i(0, reps, 1):
                    _body()
    nc.compile()
    return nc


def build_l2_nc(t_w, reps=1, dma_only=False, pe_only=False, s_ring='scalar'):
    """Launch C: paired aggregate-first with stacked W2, rel-major segs."""
    n_tiles = N_WIN * t_w
    out_ch = P
    xc = HID
    nc = bacc.Bacc(None, target_bir_lowering=False)
    G = nc.dram_tensor("G", [P, n_tiles, P], BF16, kind="ExternalInput")
    S = nc.dram_tensor("S", [P, n_tiles, SPAN], FP8, kind="ExternalInput")
    W = nc.dram_tensor("W", [P, N_REL * out_ch], BF16,
                       kind="ExternalInput")   # stacked [W2_r; W2_r]
    root = nc.dram_tensor("root", [xc, out_ch], BF16, kind="ExternalInput")
    bias = nc.dram_tensor("bias", [out_ch, 1], F32, kind="ExternalInput")
    xT = nc.dram_tensor("xT", [xc, NPC_PAD], BF16, kind="ExternalInput")
    outT = nc.dram_tensor("outT", [out_ch, NPC_PAD], BF16,
                          kind="ExternalOutput")
    with tile.TileContext(nc) as tc:
        with tc.tile_pool(name="gs", bufs=3) as gs_pool, \
             tc.tile_pool(name="wp", bufs=1) as wp, \
             tc.tile_pool(name="ap", bufs=2) as apool, \
             tc.tile_pool(name="hp", bufs=2) as hpool, \
             tc.tile_pool(name="psA", bufs=2, space="PSUM") as psA, \
             tc.tile_pool(name="psH", bufs=2, space="PSUM") as psH:
            w_t = wp.tile([P, N_REL * out_ch], BF16)
            nc.sync.dma_start(out=w_t[:], in_=W[:])
            root_t = wp.tile([xc, out_ch], BF16)
            nc.sync.dma_start(out=root_t[:], in_=root[:])
            bias_t = wp.tile([out_ch, 1], F32)
            nc.sync.dma_start(out=bias_t[:], in_=bias[:])
            xT_t = wp.tile([xc, NPC_PAD], BF16)
            nc.sync.dma_start(out=xT_t[:], in_=xT[:])

            n_grp_c = WIN // STG
            grp_tiles = {g: [] for g in range(n_grp_c)}
            for i in range(t_w):
                b = _block_base(i, t_w) // STG
                for d in range(SPAN // STG):
                    grp_tiles[b + d].append(i)

            def _body():
              for w0 in range(0, N_WIN, GG):
                nw = min(GG, N_WIN - w0)
                t0 = w0 * t_w
                g_t = gs_pool.tile([P, nw * t_w * P], BF16, tag="g")
                s_t = gs_pool.tile([P, nw * t_w * SPAN], FP8, tag="s")
                if not pe_only:
                    nc.sync.dma_start(
                        out=g_t[:], in_=G[:, t0:t0 + nw * t_w, :].rearrange(
                            "p t c -> p (t c)"))
                    s_eng = nc.scalar if s_ring == 'scalar' else nc.sync
                    s_eng.dma_start(
                        out=s_t[:], in_=S[:, t0:t0 + nw * t_w, :].rearrange(
                            "p t c -> p (t c)"))
                if dma_only:
                    nc.sync.dma_start(out=outT[:, w0 * P:(w0 + nw) * P],
                                      in_=g_t[:, :nw * P])
                    continue
                a2_t = apool.tile([P, GG, N_REL, P], BF16, tag="aev")
                for dw in range(nw):
                    w = w0 + dw
                    a_ps = psA.tile([P, WIN], F32, tag="apsum")
                    for g in range(n_grp_c):
                        tl = grp_tiles[g]
                        for idx, i in enumerate(tl):
                            j = dw * t_w + i
                            col0 = _block_base(i, t_w)
                            c_lo = g * STG - col0
                            nc.tensor.matmul(
                                out=a_ps[:, g * STG:(g + 1) * STG],
                                lhsT=g_t[:, j * P:(j + 1) * P],
                                rhs=s_t[:, j * SPAN + c_lo:
                                         j * SPAN + c_lo + STG],
                                start=(idx == 0), stop=(idx == len(tl) - 1))
                    if w % 2 == 0:
                        nc.scalar.copy(out=a2_t[:, dw], in_=a_ps[:]
                                       .rearrange("c (r n) -> c r n",
                                                  r=N_REL))
                    else:
                        nc.vector.tensor_copy(out=a2_t[:, dw], in_=a_ps[:]
                                              .rearrange("c (r n) -> c r n",
                                                         r=N_REL))
                h_ps = psH.tile([out_ch, nw * P], F32, tag="hpsum")
                for r in range(N_REL):
                    nc.tensor.matmul(
                        out=h_ps.rearrange("o (g n) -> o g n", g=nw),
                        lhsT=w_t[:, r * out_ch:(r + 1) * out_ch],
                        rhs=a2_t[:, :nw, r, :],
                        start=(r == 0), stop=False)
                nc.tensor.matmul(
                    out=h_ps[:], lhsT=root_t[:],
                    rhs=xT_t[:, w0 * P:(w0 + nw) * P],
                    start=False, stop=True)
                h_s = hpool.tile([out_ch, nw * P], BF16, tag="hev")
                nc.scalar.activation(
                    out=h_s[:], in_=h_ps[:],
                    func=mybir.ActivationFunctionType.Identity,
                    bias=bias_t[:])
                if not pe_only:
                    nc.sync.dma_start(out=outT[:, w0 * P:(w0 + nw) * P],
                                      in_=h_s[:])

            if reps == 1:
                _body()
            else:
                with tc.For_i(0, reps, 1):
                    _body()
    nc.compile()
    return nc


# ---------------- host orchestration ----------------

def _run(nc, in_maps, out_name):
    res = run_bass_kernel_spmd(nc, in_maps, list(range(N_CORES)))
    return [r[out_name] for r in res.results]


def prep_graphs(src, dst, et):
    """Per-core prep for launches B and C. Returns (coresB, t_wB,
    coresC, t_wC)."""
    gseg = dst * N_REL + et
    deg = np.bincount(gseg, minlength=N_NODES * N_REL).astype(np.float32)
    norm_all = (1.0 / np.maximum(deg[gseg], 1.0)).astype(np.float32)

    coresB, coresC = [], []
    t_wB = t_wC = 0
    for k in range(N_CORES):
        mask = (dst // NPC) == k
        e_src = src[mask]
        e_dstl = dst[mask] - k * NPC
        e_et = et[mask]
        e_norm = norm_all[mask]

        # ---- launch B: segments = dst node (window of 128 nodes) ----
        indeg = np.bincount(e_dstl, minlength=NPC)
        slotsB_node = -(-indeg // 2)
        permB = balance_windows(slotsB_node)
        invB = np.full(NPC, -1, dtype=np.int64)
        vB = permB >= 0
        invB[permB[vB]] = np.nonzero(vB)[0]
        segB = invB[e_dstl]                      # per-edge new node position
        orderB = np.argsort(segB, kind='stable')
        coresB.append(dict(order=orderB, seg=segB[orderB],
                           src=e_src[orderB], et=e_et[orderB],
                           norm=e_norm[orderB], perm=permB, inv=invB))

        # ---- launch C: rel-major (node,rel) segments ----
        slotsC_node = np.zeros(NPC, dtype=np.int64)
        degnr = np.bincount(e_dstl * N_REL + e_et, minlength=NPC * N_REL)
        slotsC_node = (-(-degnr.reshape(NPC, N_REL) // 2)).sum(1)
        permC = balance_windows(slotsC_node)
        invC = np.full(NPC, -1, dtype=np.int64)
        vC = permC >= 0
        invC[permC[vC]] = np.nonzero(vC)[0]
        nposC = invC[e_dstl]
        segC = (nposC // P) * WIN + e_et * P + (nposC % P)
        orderC = np.argsort(segC, kind='stable')
        coresC.append(dict(seg=segC[orderC], src=e_src[orderC],
                           norm=e_norm[orderC], perm=permC, inv=invC))

    # pair + slot-assign, with global t_w search
    t_wB = 0
    for c in coresB:
        so, half, sseg = pair_up(c['seg'])
        c['slot_of_edge'], c['half'], c['slot_seg'] = so, half, sseg
        wl = np.bincount(sseg // P, minlength=N_WIN).max()
        t_wB = max(t_wB, -(-int(wl) // P))
    for c in coresB:
        st, sp = prep_win_slots(c['slot_seg'], t_wB, P)
        c['st'], c['sp'] = st, sp

    # launch C pairing + greedy block assignment
    t_wC = 0
    for c in coresC:
        so, half, sseg = pair_up(c['seg'])
        c['slot_of_edge'], c['half'], c['slot_seg'] = so, half, sseg
        wl = np.bincount(sseg // WIN, minlength=N_WIN).max()
        t_wC = max(t_wC, -(-int(wl) // P))
    while True:
        ok = True
        for c in coresC:
            r = prep_win_slots(c['slot_seg'], t_wC, WIN)
            if r is None:
                ok = False
                break
            c['st'], c['sp'] = r
        if ok:
            break
        t_wC += 1
    return coresB, t_wB, coresC, t_wC


def build_GS(c, t_w, msgs, win):
    """G (paired bf16) and S (fp8 one-hot) streams for one core."""
    n_tiles = N_WIN * t_w
    G = np.zeros((P, n_tiles, P), dtype=BF)
    st_e = c['st'][c['slot_of_edge']]
    sp_e = c['sp'][c['slot_of_edge']]
    for hf in (0, 1):
        m = c['half'] == hf
        G[sp_e[m], st_e[m], hf * HID:(hf + 1) * HID] = msgs[m]
    width = P if win == P else SPAN
    S = np.zeros((P, n_tiles, width), dtype=F8)
    if win == P:
        col = c['slot_seg'] % P
    else:
        col = (c['slot_seg'] % WIN) - _block_base_vec(c['st'], t_w)
    S[c['sp'], c['st'], col] = 1.0
    return G, S


def _permuted_xT(feat_bf, perm, k, ch):
    xTk = np.zeros((ch, NPC_PAD), dtype=BF)
    valid = perm >= 0
    xTk[:, valid] = feat_bf[k * NPC + perm[valid]].T
    return xTk


def unpermute(parts, cores, ch):
    out = np.empty((N_NODES, ch), dtype=np.float32)
    for k in range(N_CORES):
        perm = cores[k]['perm']
        valid = perm >= 0
        out[k * NPC + perm[valid]] = \
            np.asarray(parts[k]).astype(np.float32).T[valid]
    return out


def kernel(x, edge_index, edge_type, W1, root1, b1, W2, root2, b2):
    x = np.asarray(x, dtype=np.float32)
    src = np.asarray(edge_index[0], dtype=np.int64)
    dst = np.asarray(edge_index[1], dtype=np.int64)
    et = np.asarray(edge_type, dtype=np.int64)
    W1 = np.asarray(W1, np.float32)
    root1 = np.asarray(root1, np.float32)
    b1 = np.asarray(b1, np.float32)
    W2 = np.asarray(W2, np.float32)
    root2 = np.asarray(root2, np.float32)
    b2 = np.asarray(b2, np.float32)

    coresB, t_wB, coresC, t_wC = prep_graphs(src, dst, et)

    # ---- launch A: xt = per-rel transform ----
    ncA = build_stage0_nc()
    x_bf = x.astype(BF)
    W1t = np.ascontiguousarray(
        W1.transpose(1, 0, 2).reshape(P, N_REL * HID)).astype(BF)
    in_mapsA = []
    for k in range(N_CORES):
        xTk = np.zeros((P, NPC_PAD), dtype=BF)
        xTk[:, :NPC] = x_bf[k * NPC:(k + 1) * NPC].T
        in_mapsA.append({"W": W1t, "xT": xTk})
    xt_parts = _run(ncA, in_mapsA, "xtO")   # [P, N_WIN, N_REL*HID] each
    # -> xt_all[core, node_local, rel, hid]
    xt_all = np.stack([
        np.asarray(p).astype(BF).transpose(1, 0, 2)
        .reshape(NPC_PAD, N_REL, HID) for p in xt_parts])

    # ---- launch B: layer-1 scatter ----
    ncB = build_l1_nc(t_wB)
    rootp = np.zeros((P, P), dtype=BF)
    rootp[:, :HID] = root1.astype(BF)
    in_mapsB = []
    for k in range(N_CORES):
        c = coresB[k]
        ks = c['src'] // NPC
        pos = c['src'] % NPC
        msgs = (xt_all[ks, pos, c['et']].astype(np.float32)
                * c['norm'][:, None]).astype(BF)
        G, S = build_GS(c, t_wB, msgs, P)
        in_mapsB.append({
            "G": G, "S": S, "rootp": rootp,
            "bias": b1.reshape(-1, 1).astype(np.float32),
            "xT": _permuted_xT(x_bf, c['perm'], k, P),
        })
    hT_parts = _run(ncB, in_mapsB, "outT")
    h = unpermute(hT_parts, coresB, HID)        # [N, 64] f32

    # ---- launch C: layer 2 ----
    ncC = build_l2_nc(t_wC)
    h_bf = h.astype(BF)
    W2t = np.ascontiguousarray(
        W2.transpose(1, 0, 2).reshape(HID, N_REL * P)).astype(BF)
    W2s = np.concatenate([W2t, W2t], axis=0)    # [128, R*128]
    in_mapsC = []
    for k in range(N_CORES):
        c = coresC[k]
        msgs = (h[c['src']] * c['norm'][:, None]).astype(BF)
        G, S = build_GS(c, t_wC, msgs, WIN)
        in_mapsC.append({
            "G": G, "S": S, "W": W2s,
            "root": root2.astype(BF),
            "bias": b2.reshape(-1, 1).astype(np.float32),
            "xT": _permuted_xT(h_bf, c['perm'], k, HID),
        })
    outT_parts = _run(ncC, in_mapsC, "outT")
    out = unpermute(outT_parts, coresC, P)
    return out.astype(np.float32)
